# revision 5
# baseline (speedup 1.0000x reference)
"""Single-head attention (B=8, S=2048, D=384) on 8 NeuronCores.

Sharding: data-parallel over batch — core b computes batch element b
entirely, weights replicated.

Host-side marshalling (layout only, zero FLOPs): x fed pre-transposed and
pre-tiled as xt [128, 3, 2048] ([d mod 128, d tile, s]); weights fed as
wq_nat/wk_nat = Wq/Wk pre-tiled [128, 3, 384] ([e mod 128, e tile, d]) and
wvt = Wv^T pre-tiled [128, 3, 384] ([d mod 128, d tile, e]).

Per-core dataflow (all on one NeuronCore, f32 in/out):
  - QK fold: scores = (x Wq^T)(x Wk^T)^T = x (Wq^T Wk) x^T. M = Wq^T Wk
    [384, 384] costs 9 small matmuls (~3.5k PE cycles) and replaces the
    separate Q and K projections (~37k cycles) with ONE projection
    TT[d', s] = sum_d M[d, d'] xT[d, s] (~18.4k cycles).
  - V = x @ Wv^T in natural [S, D] layout with a ones-column pair
    appended -> vA [S, D+2].
  - scores^T tile alphaT[k, q] = xT-block^T @ TT-block accumulated over
    the 3 d'-tiles; exp() on ScalarE (logits ~N(0,42) so fp32 exp cannot
    overflow; softmax is shift-invariant so no max subtraction needed).
  - out_raw[q, :D] and the softmax denominator accumulate together via
    out_acc[q, 0:D+2] += expT[k, qblock]^T @ vA[kblock, :] (the ones
    columns of vA make column D equal sum_k exp).
  - out[q, e] = out_raw[q, e] * (1 / out_acc[q, D]).

PE prewarm: the HAM clock gate keeps the PE at 1.2 GHz until it has been
busy ~3.4us. A run of dependency-free dummy matmuls on zeroed tiles at
kernel start (while the input DMAs stream) warms the gate so all real
matmuls run at 2.4 GHz.

Matmuls run as float32r (full PE rate at N>=256); fp32 PSUM accumulation.
"""

import os
import numpy as np

import concourse.bacc as bacc
import concourse.tile as tile
from concourse import mybir
from concourse import bass_utils

P = 128          # partitions / PE tile edge
S = 2048         # sequence length per core
D = 384          # model dim
NB = 8           # batch == number of cores
DT = D // P      # 3 feature tiles
ST = S // P      # 16 sequence tiles
QC = 512         # q-column chunk (PSUM bank of f32)
NQ = S // QC     # 4 q chunks
F32 = mybir.dt.float32
F32R = mybir.dt.float32r
BF16 = mybir.dt.bfloat16

# "f32r" (default) or "hybrid" — hybrid runs the PV path in bf16.
MM_MODE = os.environ.get("ATT_MM_MODE", "f32r")
N_WARM = int(os.environ.get("ATT_WARM", "8"))


def _build():
    sb_dt = F32R
    pv_dt = BF16 if MM_MODE == "hybrid" else F32R

    nc = bacc.Bacc(
        "TRN2", target_bir_lowering=False, debug=False, enable_asserts=False
    )
    # DRAM inputs pre-tiled on host so every load is one big DMA; dtype
    # float32r == float32 bit layout, so the direct DMA is cast-free.
    xt = nc.dram_tensor("xt", [P, DT, S], F32R, kind="ExternalInput").ap()
    wqn = nc.dram_tensor("wqn", [P, DT, D], F32R, kind="ExternalInput").ap()
    wkn = nc.dram_tensor("wkn", [P, DT, D], F32R, kind="ExternalInput").ap()
    wvt = nc.dram_tensor("wvt", [P, DT, D], F32R, kind="ExternalInput").ap()
    out = nc.dram_tensor("out", [S, D], F32, kind="ExternalOutput").ap()

    with tile.TileContext(nc) as tc:
        with (
            tc.tile_pool(name="const", bufs=1) as const_pool,
            tc.tile_pool(name="big", bufs=1) as big,
            tc.tile_pool(name="expool", bufs=4) as ex_pool,
            tc.tile_pool(name="obpool", bufs=3) as ob_pool,
            tc.tile_pool(name="smalls", bufs=4) as small_pool,
            tc.tile_pool(name="ps_stage", bufs=4, space="PSUM") as ps_stage,
            tc.tile_pool(name="ps_acc", bufs=4, space="PSUM") as ps_acc,
        ):
            ones_c = const_pool.tile([P, 2], F32, tag="ones", name="ones_c")
            nc.vector.memset(ones_c, 1.0)
            # zeroed dummy operands for the PE prewarm (memset only supports
            # plain dtypes, so zero an f32 tile and copy-cast into f32r)
            warm_w = const_pool.tile([P, P], sb_dt, tag="warmw", name="warm_w")
            warm_m = const_pool.tile([P, QC], sb_dt, tag="warmm", name="warm_m")
            warm_z = const_pool.tile([P, QC], F32, tag="warmz", name="warm_z")
            nc.vector.memset(warm_z, 0.0)
            nc.vector.tensor_copy(warm_m, warm_z)
            nc.vector.tensor_copy(warm_w, warm_z[:, 0:P])

            # Persistent per-core operands.
            xT = big.tile([P, DT, S], sb_dt, tag="xT", name="xT")
            tT = big.tile([P, DT, S], sb_dt, tag="tT", name="tT")
            # +2 ones columns: fp32r matmuls need even free sizes, so the
            # denominator column is duplicated (col D and D+1 both = 1.0)
            vA = big.tile([P, ST, D + 2], pv_dt, tag="vA", name="vA")
            wqT = big.tile([P, DT, D], sb_dt, tag="wqT", name="wqT")
            wkT = big.tile([P, DT, D], sb_dt, tag="wkT", name="wkT")
            wvT = big.tile([P, DT, D], sb_dt, tag="wvT", name="wvT")
            mT = big.tile([P, DT, D], sb_dt, tag="mT", name="mT")

            # ---- PSUM rotation across both pools (8 banks) ----------------
            _proj_n = [0]

            def proj_tile():
                _proj_n[0] += 1
                if _proj_n[0] % 2:
                    return ps_stage.tile([P, QC], F32, tag="ps1", name="pj")
                return ps_acc.tile([P, QC], F32, tag="acc", name="pj")

            # ---- PE prewarm: dependency-free dummy matmuls ----------------
            for _ in range(N_WARM):
                pw = proj_tile()
                nc.tensor.matmul(pw, warm_w, warm_m, start=True, stop=True)

            # ---- input DMAs ----------------------------------------------
            # Everything rides the sync HWDGE queue, in exactly the order
            # the PE consumes it (one busy HWDGE queue streams at the full
            # ~0.35 MB/us; the gpsimd software DGE measured 5-7x slower to
            # first byte, so it is not used at all).
            nc.sync.dma_start(out=wvT, in_=wvt)
            for lo, hi in ((0, QC // 2), (QC // 2, QC)):
                nc.sync.dma_start(out=xT[:, :, lo:hi], in_=xt[:, :, lo:hi])
            nc.sync.dma_start(out=wqT, in_=wqn)
            nc.sync.dma_start(out=wkT, in_=wkn)
            for qc in range(1, NQ):
                nc.sync.dma_start(
                    out=xT[:, :, qc * QC:(qc + 1) * QC],
                    in_=xt[:, :, qc * QC:(qc + 1) * QC],
                )

            # ---- projections ---------------------------------------------
            def project_v(st):
                # V natural: V[s, e] = sum_d xT[d, s] * WvT[d, e]
                pv = proj_tile()
                for dt_ in range(DT):
                    nc.tensor.matmul(
                        pv[:, 0:D],
                        xT[:, dt_, st * P:(st + 1) * P],
                        wvT[:, dt_, :],
                        start=(dt_ == 0),
                        stop=(dt_ == DT - 1),
                    )
                nc.vector.tensor_copy(vA[:, st, 0:D], pv[:, 0:D])

            def compute_m():
                # M[d, d'] = sum_e Wq[e, d] Wk[e, d']
                for dt_ in range(DT):
                    pm = proj_tile()
                    for et in range(DT):
                        nc.tensor.matmul(
                            pm[:, 0:D],
                            wqT[:, et, dt_ * P:(dt_ + 1) * P],
                            wkT[:, et, :],
                            start=(et == 0),
                            stop=(et == DT - 1),
                        )
                    nc.vector.tensor_copy(mT[:, dt_, :], pm[:, 0:D])

            # TT feature-major: TT[d', s] = sum_d M[d, d'] xT[d, s]
            def project_t_chunk(qc, et):
                pp = proj_tile()
                for dt_ in range(DT):
                    nc.tensor.matmul(
                        pp,
                        mT[:, dt_, et * P:(et + 1) * P],
                        xT[:, dt_, qc * QC:(qc + 1) * QC],
                        start=(dt_ == 0),
                        stop=(dt_ == DT - 1),
                    )
                nc.vector.tensor_copy(tT[:, et, qc * QC:(qc + 1) * QC], pp)

            # V rows of chunk 0 first (needs only wv + x0), then M (needs
            # wq + wk), then per chunk: T columns + V rows — matching the
            # DMA arrival order so the PE never waits past the first chunk.
            for st in range(4):
                project_v(st)
            compute_m()
            for et in range(DT):
                project_t_chunk(0, et)
            for qc in range(1, NQ):
                for st in range(qc * 4, qc * 4 + 4):
                    project_v(st)
                for et in range(DT):
                    project_t_chunk(qc, et)
            # ones columns for every V row tile in one strided copy
            nc.vector.tensor_copy(
                vA[:, :, D:D + 2],
                ones_c.unsqueeze(1).broadcast_to([P, ST, 2]),
            )

            # ---- attention, one 512-wide q chunk at a time ----------------
            for c in range(NQ):
                accs = [
                    ps_acc.tile([P, D + 2], F32, tag="acc", name="acc")
                    for _ in range(4)
                ]

                def emit_pv(kt_i, ex):
                    for qs in range(4):
                        nc.tensor.matmul(
                            accs[qs],
                            ex[:, qs * P:(qs + 1) * P],
                            vA[:, kt_i, :],
                            start=(kt_i == 0),
                            stop=(kt_i == ST - 1),
                        )

                pending = []
                for kt_i in range(ST):
                    pa = ps_stage.tile([P, QC], F32, tag="ps1", name="pa")
                    for et in range(DT):
                        nc.tensor.matmul(
                            pa,
                            xT[:, et, kt_i * P:(kt_i + 1) * P],
                            tT[:, et, c * QC:(c + 1) * QC],
                            start=(et == 0),
                            stop=(et == DT - 1),
                        )
                    ex = ex_pool.tile([P, QC], pv_dt, tag="ex", name="ex")
                    nc.scalar.activation(
                        ex, pa, mybir.ActivationFunctionType.Exp
                    )
                    # software-pipeline PV two k-tiles behind the QK+exp so
                    # the PE never waits on a just-issued exp
                    pending.append((kt_i, ex))
                    if len(pending) > 2:
                        emit_pv(*pending.pop(0))
                for item in pending:
                    emit_pv(*item)

                # epilogue split across DVE and ACT so the tail chain halves;
                # all reciprocals first so the ACT-side muls never wait on a
                # reciprocal queued behind a DVE mul
                recs = []
                for qs in range(4):
                    rec = small_pool.tile([P, 1], F32, tag="rec", name="rec")
                    nc.vector.reciprocal(rec, accs[qs][:, D:D + 1])
                    recs.append(rec)
                for qs in range(4):
                    ob = ob_pool.tile([P, D], F32, tag="ob", name="ob")
                    qt_row = (c * 4 + qs) * P
                    if qs % 2:
                        nc.scalar.activation(
                            ob,
                            accs[qs][:, 0:D],
                            mybir.ActivationFunctionType.Copy,
                            scale=recs[qs],
                        )
                        nc.scalar.dma_start(
                            out=out[qt_row:qt_row + P, :], in_=ob
                        )
                    else:
                        nc.vector.tensor_scalar_mul(
                            ob, accs[qs][:, 0:D], recs[qs]
                        )
                        nc.sync.dma_start(
                            out=out[qt_row:qt_row + P, :], in_=ob
                        )

    nc.compile()
    return nc


_NC = None
_FAST = None


def _get_nc():
    global _NC
    if _NC is None:
        _NC = _build()
    return _NC


def _fast_runner():
    """Build (once) a jitted shard_map callable over the 8 cores."""
    global _FAST
    if _FAST is not None:
        return _FAST
    import jax
    from jax.experimental.shard_map import shard_map
    from jax.sharding import Mesh, PartitionSpec

    from concourse import bass2jax

    nc = _get_nc()
    bass2jax.install_neuronx_cc_hook()

    in_names = ["xt", "wqn", "wkn", "wvt"]
    out_aval = jax.core.ShapedArray((S, D), np.float32)

    def _body(*args):
        operands = list(args)
        operands.append(bass2jax.partition_id_tensor())
        outs = bass2jax._bass_exec_p.bind(
            *operands,
            out_avals=(out_aval,),
            in_names=tuple(in_names) + ("out", "partition_id"),
            out_names=("out",),
            lowering_input_output_aliases=(),
            sim_require_finite=True,
            sim_require_nnan=True,
            nc=nc,
        )
        return tuple(outs)

    devices = jax.devices()[:NB]
    mesh = Mesh(np.asarray(devices), ("core",))
    n_in = len(in_names) + 1  # + donated zero output
    fn = jax.jit(
        shard_map(
            _body,
            mesh=mesh,
            in_specs=(PartitionSpec("core"),) * n_in,
            out_specs=(PartitionSpec("core"),),
            check_rep=False,
        ),
        donate_argnums=(n_in - 1,),
        keep_unused=True,
    )
    _FAST = fn
    return fn


def _tile_ed(w):
    # [384, x] -> [128, 3, x] pre-tiled partition-major (layout only)
    return np.ascontiguousarray(
        w.reshape(DT, P, w.shape[1]).transpose(1, 0, 2)
    )


def _marshal(att_input, Wq, Wk, Wv):
    att_input = np.asarray(att_input, dtype=np.float32)
    # per-core x: [D, S] -> pre-tiled [128, 3, S]
    xts = np.ascontiguousarray(
        att_input.transpose(0, 2, 1)
        .reshape(NB, DT, P, S)
        .transpose(0, 2, 1, 3)
    )  # [NB, 128, 3, S]
    wq = _tile_ed(np.asarray(Wq, dtype=np.float32))            # Wq [e,d]
    wk = _tile_ed(np.asarray(Wk, dtype=np.float32))            # Wk [e,d]
    wv = _tile_ed(np.ascontiguousarray(np.asarray(Wv, np.float32).T))
    return xts, (wq, wk, wv)


def run(att_input, Wq, Wk, Wv, trace=False):
    xts, wts = _marshal(att_input, Wq, Wk, Wv)
    if trace:
        in_maps = [
            {"xt": xts[b], "wqn": wts[0], "wkn": wts[1], "wvt": wts[2]}
            for b in range(NB)
        ]
        res = bass_utils.run_bass_kernel_spmd(
            _get_nc(), in_maps, core_ids=list(range(NB)), trace=True
        )
        out = np.stack([res.results[b]["out"] for b in range(NB)], axis=0)
        return out.astype(np.float32, copy=False), res

    try:
        fn = _fast_runner()
        xs = xts.reshape(NB * P, DT, S)
        ws = [
            np.concatenate([w] * NB, axis=0).reshape(NB * P, DT, D)
            for w in wts
        ]
        zeros = np.zeros((NB * S, D), np.float32)
        (out,) = fn(xs, *ws, zeros)
        out = np.asarray(out)
    except Exception:
        in_maps = [
            {"xt": xts[b], "wqn": wts[0], "wkn": wts[1], "wvt": wts[2]}
            for b in range(NB)
        ]
        res = bass_utils.run_bass_kernel_spmd(
            _get_nc(), in_maps, core_ids=list(range(NB))
        )
        out = np.stack([res.results[b]["out"] for b in range(NB)], axis=0)
    return out.reshape(NB, S, D).astype(np.float32, copy=False), None


def kernel(att_input, Wq, Wk, Wv):
    out, _ = run(att_input, Wq, Wk, Wv)
    return out
